# revision 10
# baseline (speedup 1.0000x reference)
"""Trainium2 Bass kernel for AttentionRNN (Bahdanau-style additive attention).

Reference computation (B=32, T=2048, D=U=1024):
    q_proj = (query @ w1 + b1)[:, None, :]          # [B, 1, U]
    k_proj = values @ w2 + b2                        # [B, T, U]
    score  = tanh(q_proj + k_proj) @ v + bv          # [B, T, 1]
    attention_weights = softmax(score, axis=1)       # [B, T, 1]
    context = sum(attention_weights * values, axis=1)  # [B, D]
    returns (context, attention_weights)

Sharding: batch B split across 8 NeuronCores (4 examples/core); w2/v and the
q-projection (computed on host, it is tiny) are replicated. Per core the
dominant work is k_proj = values @ w2 (17.2 GFLOP) done in bf16 on the PE,
with values transposed on-chip via the XBAR DMA-transpose so the contraction
dim (d) lands on partitions. bv shifts all scores uniformly -> softmax
invariant -> dropped.

The XBAR transpose of one [128, 4x1024] input lands as out[d%128, s*8+dt, t]
(s = t-slab, dt = d-tile), so a quarter example transposes in ONE
DMA_TRANSPOSE instruction (issue cost ~1.3us on the dispatching engine) and
the matmul moving operand reads it with a strided AP. Big loads + XBARs
share the sync HWDGE queue (in dependency order); small softmax/output DMAs
go via SWDGE (gpsimd) so nothing blocks the load/transpose pipeline.
"""

import numpy as np
import ml_dtypes

B, T, D, U = 32, 2048, 1024, 1024
N_CORES = 8
BL = B // N_CORES          # 4 examples per core
P = 128
NT = T // P                # 16 t-slabs of 128 per example
TC = 512                   # t-chunk for the main matmul
NCH = T // TC              # 4 chunks per example
SLABS_PER_CHUNK = TC // P  # 4
DT = D // P                # 8 d-tiles
UT = U // P                # 8 u-tiles
LOAD_GROUP = 4             # t-slabs per HBM load DMA (1 MB each)

_cache = {}


def _build():
    import concourse.bass as bass
    import concourse.mybir as mybir
    import concourse.tile as tile
    from concourse import bacc

    f32 = mybir.dt.float32
    bf16 = mybir.dt.bfloat16
    AF = mybir.ActivationFunctionType

    nc = bacc.Bacc("TRN2", target_bir_lowering=False, debug=False,
                   num_devices=N_CORES)

    # Per-core inputs (values pre-cast to bf16 on host)
    vals = nc.dram_tensor("vals", [BL, T, D], bf16, kind="ExternalInput")
    w2s = nc.dram_tensor("w2s", [P, UT * DT * P], bf16, kind="ExternalInput")
    qbt = nc.dram_tensor("qbt", [P, UT * BL], f32, kind="ExternalInput")
    v8 = nc.dram_tensor("v8", [P, UT], bf16, kind="ExternalInput")
    # Per-core outputs
    ctx_o = nc.dram_tensor("ctx", [BL, D], f32, kind="ExternalOutput")
    aw_o = nc.dram_tensor("aw", [BL, T], f32, kind="ExternalOutput")
    vals_r = vals.ap().rearrange("b (n p) d -> b n p d", p=P)  # [BL, NT, 128, D]
    w2s_r = w2s.ap().rearrange("p (a b c) -> p a (b c)", b=DT, c=P)  # [P, UT, DT*P]

    with tile.TileContext(nc) as tc:
        with (
            tc.tile_pool(name="consts", bufs=1) as consts,
            tc.tile_pool(name="v16", bufs=2) as v16_p,
            tc.tile_pool(name="vt", bufs=2) as vt_p,
            tc.tile_pool(name="tanh", bufs=8) as tanh_p,
            tc.tile_pool(name="sc", bufs=1) as sc_p,
            tc.tile_pool(name="small", bufs=3) as small_p,
            tc.tile_pool(name="w16", bufs=2) as w16_p,
            tc.tile_pool(name="wst", bufs=2) as wst_p,
            tc.tile_pool(name="cv", bufs=2) as cv_p,
            tc.tile_pool(name="psK", bufs=3, space="PSUM") as psK_p,
            tc.tile_pool(name="psS", bufs=2, space="PSUM") as psS_p,
            tc.tile_pool(name="psC", bufs=1, space="PSUM") as psC_p,
        ):
            qb_sb = consts.tile([P, UT * BL], f32)
            nc.scalar.dma_start(out=qb_sb, in_=qbt.ap())
            qb_v = qb_sb.rearrange("p (a b) -> p a b", b=BL)
            v8_sb = consts.tile([P, UT], bf16)
            nc.scalar.dma_start(out=v8_sb, in_=v8.ap())
            w2_sb = consts.tile([P, UT, DT * P], bf16)
            w2_v = w2_sb.rearrange("p a (b c) -> p a b c", c=P)  # [P,UT,DT,128]
            def load_group(v16, b, g):
                src = vals_r[b, g * LOAD_GROUP:(g + 1) * LOAD_GROUP]
                nc.scalar.dma_start(
                    out=v16[:, g * LOAD_GROUP:(g + 1) * LOAD_GROUP, :],
                    in_=src.rearrange("n p d -> p n d"))

            def xbar_quarter(vt, v16, q):
                # [128t, 4096] -> vt[:, q*32:(q+1)*32, :] with free layout
                # (s*8+dt, t) ; one DMA_TRANSPOSE instruction
                nc.sync.dma_start(out=vt[:, q * 32:(q + 1) * 32, :],
                                  in_=v16[:, q * 4:(q + 1) * 4, :],
                                  transpose=True)

            def load_xbar_example(b):
                """interleaved load-group + transpose-quarter pairs"""
                v16 = v16_p.tile([P, NT, D], bf16, tag="v16")
                vt = vt_p.tile([P, NT * DT, P], bf16, tag="vt")
                for g in range(NT // LOAD_GROUP):
                    load_group(v16, b, g)
                    xbar_quarter(vt, v16, g)
                return v16, vt

            def compute_a(b, v16, vt, prefetch, post_c0):
                """scores + softmax for example b; returns rinv.
                `prefetch` emits next-example load/xbar work mid-example;
                `post_c0` emits the previous example's context phase after
                chunk 0 so its v16 slot frees early."""
                vt_r = vt.rearrange("p (s j) t -> p s j t", j=DT)
                sc = sc_p.tile([1, T], f32)
                mx4 = small_p.tile([1, 4], f32)
                for c in range(NCH):
                    if c == 1:
                        post_c0()
                        prefetch()
                    pS = psS_p.tile([1, TC], f32)
                    ths = []
                    for ut in range(UT):
                        pK = psK_p.tile([P, TC], f32)
                        for dt in range(DT):
                            nc.tensor.matmul(
                                pK, w2_v[:, ut, dt, :],
                                vt_r[:, c * SLABS_PER_CHUNK:
                                     (c + 1) * SLABS_PER_CHUNK, dt, :],
                                start=(dt == 0), stop=(dt == DT - 1))
                        th = tanh_p.tile([P, TC], bf16)
                        nc.scalar.activation(th, pK, AF.Tanh,
                                             bias=qb_v[:, ut, b:b + 1])
                        ths.append(th)
                    for ut in range(UT):
                        nc.tensor.matmul(pS, v8_sb[:, ut:ut + 1], ths[ut],
                                         start=(ut == 0), stop=(ut == UT - 1))
                    nc.scalar.copy(sc[:, c * TC:(c + 1) * TC], pS)
                    # eager per-chunk max (off the critical path except c3)
                    nc.vector.tensor_reduce(
                        mx4[:, c:c + 1], sc[:, c * TC:(c + 1) * TC],
                        axis=mybir.AxisListType.X, op=mybir.AluOpType.max)

                # softmax over T (all on partition 0)
                m = small_p.tile([1, 1], f32)
                nc.vector.tensor_reduce(m, mx4, axis=mybir.AxisListType.X,
                                        op=mybir.AluOpType.max, negate=True)
                den = small_p.tile([1, 1], f32)
                nc.scalar.activation(sc, sc, AF.Exp, bias=m, accum_out=den)
                rinv = small_p.tile([1, 1], f32)
                nc.vector.reciprocal(rinv, den)
                # w16: row 0 = unnormalized exp weights (bf16); rows 1-15 are
                # zeroed padding so the XBAR transpose (needs 16 partitions)
                # can read the tile directly -- no DRAM roundtrip
                w16 = w16_p.tile([16, T], bf16)
                nc.vector.memset(w16, 0.0)
                nc.vector.tensor_copy(w16[0:1, :], sc)
                awn = sc_p.tile([1, T], f32, tag="awn", bufs=1)
                nc.scalar.mul(awn, sc, rinv)            # normalize on ACT (|| cast)
                nc.gpsimd.dma_start(out=aw_o.ap()[b:b + 1, :], in_=awn)
                return rinv, w16

            def phase_b(b, v16, rinv, w16):
                """context vector for example b."""
                wst = wst_p.tile([P, NT, 16], bf16)
                nc.sync.dma_start(out=wst, in_=w16, transpose=True)
                pc0 = psC_p.tile([1, 512], f32, tag="pc0")
                pc1 = psC_p.tile([1, 512], f32, tag="pc1")
                for s in range(NT):
                    nc.tensor.matmul(pc0, wst[:, s, 0:1], v16[:, s, 0:512],
                                     start=(s == 0), stop=(s == NT - 1))
                    nc.tensor.matmul(pc1, wst[:, s, 0:1], v16[:, s, 512:1024],
                                     start=(s == 0), stop=(s == NT - 1))
                cv = cv_p.tile([1, D], f32)
                nc.scalar.mul(cv[:, 0:512], pc0, rinv)
                nc.scalar.mul(cv[:, 512:1024], pc1, rinv)
                nc.gpsimd.dma_start(out=ctx_o.ap()[b:b + 1, :], in_=cv)

            # -- startup: interleave example 0's loads/transposes with w2 so
            # the first matmul can start ~8us in
            v16s = [None] * BL
            vts = [None] * BL
            v16s[0] = v16_p.tile([P, NT, D], bf16, name="v16", tag="v16")
            vts[0] = vt_p.tile([P, NT * DT, P], bf16, name="vt", tag="vt")
            nc.scalar.dma_start(out=w2_sb[:, 0, :], in_=w2s_r[:, 0, :])
            load_group(v16s[0], 0, 0)
            xbar_quarter(vts[0], v16s[0], 0)
            for ut in range(1, UT):
                nc.scalar.dma_start(out=w2_sb[:, ut, :], in_=w2s_r[:, ut, :])
            for g in range(1, NT // LOAD_GROUP):
                load_group(v16s[0], 0, g)
                xbar_quarter(vts[0], v16s[0], g)

            # pipeline: B(b-1) is emitted after chunk 0 of A(b) (frees its
            # v16 slot early), next example's loads+transposes right after
            pend = []
            for b in range(BL):
                def prefetch(b=b):
                    if b + 1 < BL:
                        v16s[b + 1], vts[b + 1] = load_xbar_example(b + 1)

                def post_c0(b=b):
                    if pend and b < BL - 1:
                        phase_b(*pend.pop(0))
                rinv, w16 = compute_a(b, v16s[b], vts[b], prefetch, post_c0)
                pend.append((b, v16s[b], rinv, w16))
            while pend:
                phase_b(*pend.pop(0))

    nc.compile()
    return nc


def _get_nc():
    if "nc" not in _cache:
        _cache["nc"] = _build()
    return _cache["nc"]


def kernel(query, values, w1, b1, w2, b2, v, bv):
    from concourse.bass_utils import run_bass_kernel_spmd

    query = np.asarray(query, np.float32)
    values = np.asarray(values, np.float32)
    w1 = np.asarray(w1, np.float32)
    b1 = np.asarray(b1, np.float32)
    w2 = np.asarray(w2, np.float32)
    b2 = np.asarray(b2, np.float32)
    v = np.asarray(v, np.float32)
    # bv only shifts scores uniformly -> softmax output unchanged; dropped.

    bf = ml_dtypes.bfloat16
    # host prep (tiny except the values cast): q-projection folded with both
    # biases, weight relayouts, values -> bf16
    qb = query @ w1 + b1 + b2                                   # [B, U]
    qbt_full = np.ascontiguousarray(
        qb.T.reshape(UT, P, B).transpose(1, 0, 2))              # [128, UT, B]
    # w2 laid out [p, ut, dt, c] so each ut-slice is one contiguous DMA
    w2s = np.ascontiguousarray(
        w2.reshape(DT, P, UT, P).transpose(1, 2, 0, 3)
        .reshape(P, UT * DT * P).astype(bf))                    # [128, UT*DT*128]
    v8 = np.ascontiguousarray(v[:, 0].reshape(UT, P).T.astype(bf))  # [128, UT]
    vals16 = np.ascontiguousarray(values.astype(bf))            # [B, T, D]

    nc = _get_nc()
    in_maps = []
    for i in range(N_CORES):
        bs = slice(i * BL, (i + 1) * BL)
        in_maps.append({
            "vals": vals16[bs],
            "w2s": w2s,
            "qbt": np.ascontiguousarray(
                qbt_full[:, :, bs].reshape(P, UT * BL)),
            "v8": v8,
        })
    res = run_bass_kernel_spmd(nc, in_maps, core_ids=list(range(N_CORES)),
                               **_cache.get("run_kwargs", {}))
    _cache["last_results"] = res

    context = np.concatenate([res.results[i]["ctx"] for i in range(N_CORES)], 0)
    aw = np.concatenate([res.results[i]["aw"] for i in range(N_CORES)], 0)
    return context.astype(np.float32), aw.reshape(B, T, 1).astype(np.float32)


# revision 11
# speedup vs baseline: 1.3260x; 1.3260x over previous
"""Trainium2 Bass kernel for AttentionRNN (Bahdanau-style additive attention).

Reference computation (B=32, T=2048, D=U=1024):
    q_proj = (query @ w1 + b1)[:, None, :]          # [B, 1, U]
    k_proj = values @ w2 + b2                        # [B, T, U]
    score  = tanh(q_proj + k_proj) @ v + bv          # [B, T, 1]
    attention_weights = softmax(score, axis=1)       # [B, T, 1]
    context = sum(attention_weights * values, axis=1)  # [B, D]
    returns (context, attention_weights)

Sharding: batch B split across 8 NeuronCores (4 examples/core); w2/v and the
q-projection (computed on host, it is tiny) are replicated. Per core the
dominant work is k_proj = values @ w2 (17.2 GFLOP) done in bf16 on the PE,
with values transposed on-chip via the XBAR DMA-transpose so the contraction
dim (d) lands on partitions. bv shifts all scores uniformly -> softmax
invariant -> dropped.

The XBAR transpose of one [128, 4x1024] input lands as out[d%128, s*8+dt, t]
(s = t-slab, dt = d-tile), so a quarter example transposes in ONE
DMA_TRANSPOSE instruction (issue cost ~1.3us on the dispatching engine) and
the matmul moving operand reads it with a strided AP. Big loads + XBARs
share the sync HWDGE queue (in dependency order); small softmax/output DMAs
go via SWDGE (gpsimd) so nothing blocks the load/transpose pipeline.
"""

import numpy as np
import ml_dtypes

B, T, D, U = 32, 2048, 1024, 1024
N_CORES = 8
BL = B // N_CORES          # 4 examples per core
P = 128
NT = T // P                # 16 t-slabs of 128 per example
TC = 512                   # t-chunk for the main matmul
NCH = T // TC              # 4 chunks per example
SLABS_PER_CHUNK = TC // P  # 4
DT = D // P                # 8 d-tiles
UT = U // P                # 8 u-tiles
LOAD_GROUP = 4             # t-slabs per HBM load DMA (1 MB each)

_cache = {}


def _build():
    import concourse.bass as bass
    import concourse.mybir as mybir
    import concourse.tile as tile
    from concourse import bacc

    f32 = mybir.dt.float32
    bf16 = mybir.dt.bfloat16
    AF = mybir.ActivationFunctionType

    nc = bacc.Bacc("TRN2", target_bir_lowering=False, debug=False,
                   num_devices=N_CORES)

    # Per-core inputs (values pre-cast to bf16 on host)
    vals = nc.dram_tensor("vals", [BL, T, D], bf16, kind="ExternalInput")
    w2s = nc.dram_tensor("w2s", [P, UT * DT * P], bf16, kind="ExternalInput")
    qbt = nc.dram_tensor("qbt", [P, UT * BL], f32, kind="ExternalInput")
    v8 = nc.dram_tensor("v8", [P, UT], bf16, kind="ExternalInput")
    # Per-core outputs
    ctx_o = nc.dram_tensor("ctx", [BL, D], f32, kind="ExternalOutput")
    aw_o = nc.dram_tensor("aw", [BL, T], f32, kind="ExternalOutput")
    vals_r = vals.ap().rearrange("b (n p) d -> b n p d", p=P)  # [BL, NT, 128, D]
    w2s_r = w2s.ap().rearrange("p (a b c) -> p a (b c)", b=DT, c=P)  # [P, UT, DT*P]

    with tile.TileContext(nc) as tc:
        with (
            tc.tile_pool(name="consts", bufs=1) as consts,
            tc.tile_pool(name="v16", bufs=2) as v16_p,
            tc.tile_pool(name="vt", bufs=2) as vt_p,
            tc.tile_pool(name="tanh", bufs=8) as tanh_p,
            tc.tile_pool(name="sc", bufs=1) as sc_p,
            tc.tile_pool(name="small", bufs=3) as small_p,
            tc.tile_pool(name="w16", bufs=2) as w16_p,
            tc.tile_pool(name="wst", bufs=2) as wst_p,
            tc.tile_pool(name="cv", bufs=2) as cv_p,
            tc.tile_pool(name="psK", bufs=3, space="PSUM") as psK_p,
            tc.tile_pool(name="psS", bufs=2, space="PSUM") as psS_p,
            tc.tile_pool(name="psC", bufs=1, space="PSUM") as psC_p,
        ):
            qb_sb = consts.tile([P, UT * BL], f32)
            nc.sync.dma_start(out=qb_sb, in_=qbt.ap())
            qb_v = qb_sb.rearrange("p (a b) -> p a b", b=BL)
            v8_sb = consts.tile([P, UT], bf16)
            nc.sync.dma_start(out=v8_sb, in_=v8.ap())
            w2_sb = consts.tile([P, UT, DT * P], bf16)
            w2_v = w2_sb.rearrange("p a (b c) -> p a b c", c=P)  # [P,UT,DT,128]
            def load_group(v16, b, g):
                src = vals_r[b, g * LOAD_GROUP:(g + 1) * LOAD_GROUP]
                nc.sync.dma_start(
                    out=v16[:, g * LOAD_GROUP:(g + 1) * LOAD_GROUP, :],
                    in_=src.rearrange("n p d -> p n d"))

            def xbar_quarter(vt, v16, q):
                # [128t, 4096] -> vt[:, q*32:(q+1)*32, :] with free layout
                # (s*8+dt, t) ; one DMA_TRANSPOSE instruction
                nc.sync.dma_start(out=vt[:, q * 32:(q + 1) * 32, :],
                                  in_=v16[:, q * 4:(q + 1) * 4, :],
                                  transpose=True)

            def load_xbar_example(b):
                """interleaved load-group + transpose-quarter pairs"""
                v16 = v16_p.tile([P, NT, D], bf16, tag="v16")
                vt = vt_p.tile([P, NT * DT, P], bf16, tag="vt")
                for g in range(NT // LOAD_GROUP):
                    load_group(v16, b, g)
                    xbar_quarter(vt, v16, g)
                return v16, vt

            def compute_a(b, v16, vt, prefetch, post_c0):
                """scores + softmax for example b; returns rinv.
                `prefetch` emits next-example load/xbar work mid-example;
                `post_c0` emits the previous example's context phase after
                chunk 0 so its v16 slot frees early."""
                vt_r = vt.rearrange("p (s j) t -> p s j t", j=DT)
                sc = sc_p.tile([1, T], f32)
                mx4 = small_p.tile([1, 4], f32)
                for c in range(NCH):
                    if c == 1:
                        post_c0()
                        prefetch()
                    pS = psS_p.tile([1, TC], f32)
                    ths = []
                    for ut in range(UT):
                        pK = psK_p.tile([P, TC], f32)
                        for dt in range(DT):
                            nc.tensor.matmul(
                                pK, w2_v[:, ut, dt, :],
                                vt_r[:, c * SLABS_PER_CHUNK:
                                     (c + 1) * SLABS_PER_CHUNK, dt, :],
                                start=(dt == 0), stop=(dt == DT - 1))
                        th = tanh_p.tile([P, TC], bf16)
                        nc.scalar.activation(th, pK, AF.Tanh,
                                             bias=qb_v[:, ut, b:b + 1])
                        ths.append(th)
                    for ut in range(UT):
                        nc.tensor.matmul(pS, v8_sb[:, ut:ut + 1], ths[ut],
                                         start=(ut == 0), stop=(ut == UT - 1))
                    nc.scalar.copy(sc[:, c * TC:(c + 1) * TC], pS)
                    # eager per-chunk max (off the critical path except c3)
                    nc.vector.tensor_reduce(
                        mx4[:, c:c + 1], sc[:, c * TC:(c + 1) * TC],
                        axis=mybir.AxisListType.X, op=mybir.AluOpType.max)

                # softmax over T (all on partition 0)
                m = small_p.tile([1, 1], f32)
                nc.vector.tensor_reduce(m, mx4, axis=mybir.AxisListType.X,
                                        op=mybir.AluOpType.max, negate=True)
                den = small_p.tile([1, 1], f32)
                nc.scalar.activation(sc, sc, AF.Exp, bias=m, accum_out=den)
                rinv = small_p.tile([1, 1], f32)
                nc.vector.reciprocal(rinv, den)
                # w16: row 0 = unnormalized exp weights (bf16); rows 1-15 are
                # zeroed padding so the XBAR transpose (needs 16 partitions)
                # can read the tile directly -- no DRAM roundtrip
                w16 = w16_p.tile([16, T], bf16)
                nc.vector.memset(w16, 0.0)
                nc.vector.tensor_copy(w16[0:1, :], sc)
                awn = sc_p.tile([1, T], f32, tag="awn", bufs=1)
                nc.scalar.mul(awn, sc, rinv)            # normalize on ACT (|| cast)
                nc.gpsimd.dma_start(out=aw_o.ap()[b:b + 1, :], in_=awn)
                return rinv, w16

            def phase_b(b, v16, rinv, w16):
                """context vector for example b."""
                wst = wst_p.tile([P, NT, 16], bf16)
                nc.sync.dma_start(out=wst, in_=w16, transpose=True)
                pc0 = psC_p.tile([1, 512], f32, tag="pc0")
                pc1 = psC_p.tile([1, 512], f32, tag="pc1")
                for s in range(NT):
                    nc.tensor.matmul(pc0, wst[:, s, 0:1], v16[:, s, 0:512],
                                     start=(s == 0), stop=(s == NT - 1))
                    nc.tensor.matmul(pc1, wst[:, s, 0:1], v16[:, s, 512:1024],
                                     start=(s == 0), stop=(s == NT - 1))
                cv = cv_p.tile([1, D], f32)
                nc.scalar.mul(cv[:, 0:512], pc0, rinv)
                nc.scalar.mul(cv[:, 512:1024], pc1, rinv)
                nc.gpsimd.dma_start(out=ctx_o.ap()[b:b + 1, :], in_=cv)

            # -- startup: interleave example 0's loads/transposes with w2 so
            # the first matmul can start ~8us in
            v16s = [None] * BL
            vts = [None] * BL
            v16s[0] = v16_p.tile([P, NT, D], bf16, name="v16", tag="v16")
            vts[0] = vt_p.tile([P, NT * DT, P], bf16, name="vt", tag="vt")
            nc.sync.dma_start(out=w2_sb[:, 0, :], in_=w2s_r[:, 0, :])
            load_group(v16s[0], 0, 0)
            xbar_quarter(vts[0], v16s[0], 0)
            for ut in range(1, UT):
                nc.sync.dma_start(out=w2_sb[:, ut, :], in_=w2s_r[:, ut, :])
            for g in range(1, NT // LOAD_GROUP):
                load_group(v16s[0], 0, g)
                xbar_quarter(vts[0], v16s[0], g)

            # pipeline: B(b-1) is emitted after chunk 0 of A(b) (frees its
            # v16 slot early), next example's loads+transposes right after
            pend = []
            for b in range(BL):
                def prefetch(b=b):
                    if b + 1 < BL:
                        v16s[b + 1], vts[b + 1] = load_xbar_example(b + 1)

                def post_c0(b=b):
                    if pend and b < BL - 1:
                        phase_b(*pend.pop(0))
                rinv, w16 = compute_a(b, v16s[b], vts[b], prefetch, post_c0)
                pend.append((b, v16s[b], rinv, w16))
            while pend:
                phase_b(*pend.pop(0))

    nc.compile()
    return nc


def _get_nc():
    if "nc" not in _cache:
        _cache["nc"] = _build()
    return _cache["nc"]


def kernel(query, values, w1, b1, w2, b2, v, bv):
    from concourse.bass_utils import run_bass_kernel_spmd

    query = np.asarray(query, np.float32)
    values = np.asarray(values, np.float32)
    w1 = np.asarray(w1, np.float32)
    b1 = np.asarray(b1, np.float32)
    w2 = np.asarray(w2, np.float32)
    b2 = np.asarray(b2, np.float32)
    v = np.asarray(v, np.float32)
    # bv only shifts scores uniformly -> softmax output unchanged; dropped.

    bf = ml_dtypes.bfloat16
    # host prep (tiny except the values cast): q-projection folded with both
    # biases, weight relayouts, values -> bf16
    qb = query @ w1 + b1 + b2                                   # [B, U]
    qbt_full = np.ascontiguousarray(
        qb.T.reshape(UT, P, B).transpose(1, 0, 2))              # [128, UT, B]
    # w2 laid out [p, ut, dt, c] so each ut-slice is one contiguous DMA
    w2s = np.ascontiguousarray(
        w2.reshape(DT, P, UT, P).transpose(1, 2, 0, 3)
        .reshape(P, UT * DT * P).astype(bf))                    # [128, UT*DT*128]
    v8 = np.ascontiguousarray(v[:, 0].reshape(UT, P).T.astype(bf))  # [128, UT]
    vals16 = np.ascontiguousarray(values.astype(bf))            # [B, T, D]

    nc = _get_nc()
    in_maps = []
    for i in range(N_CORES):
        bs = slice(i * BL, (i + 1) * BL)
        in_maps.append({
            "vals": vals16[bs],
            "w2s": w2s,
            "qbt": np.ascontiguousarray(
                qbt_full[:, :, bs].reshape(P, UT * BL)),
            "v8": v8,
        })
    res = run_bass_kernel_spmd(nc, in_maps, core_ids=list(range(N_CORES)),
                               **_cache.get("run_kwargs", {}))
    _cache["last_results"] = res

    context = np.concatenate([res.results[i]["ctx"] for i in range(N_CORES)], 0)
    aw = np.concatenate([res.results[i]["aw"] for i in range(N_CORES)], 0)
    return context.astype(np.float32), aw.reshape(B, T, 1).astype(np.float32)
